# revision 11
# baseline (speedup 1.0000x reference)
"""Trainium2 Bass kernel for the 2-layer LSTM LM problem.

Strategy: tensor-parallel over gate columns across 8 cores, with the
layer-1 computation skewed one timestep behind layer 0 so the two
h-slice exchanges per step collapse into ONE AllGather per phase.

  - Each core owns a 128-wide slice of each gate (f,i,o,g) for both LSTM
    layers -> per-core weight slices [K, 512] stay SBUF-resident (bf16).
  - All matmuls run in bf16 (1 cyc/row on the PE vs 4 for fp32), fp32
    PSUM accumulation; gate math / cell state stay fp32.
  - Phase t computes L0 step t and L1 step t-1.  The h0T(t) and
    h1T(t-1) transposed slices are packed into one [128, 256] bf16 tile
    and AllGathered once per phase.  The gather result is block-copied
    into SBUF slots hT[:, r*256:(r+1)*256] = rank r's [h0T | h1T]
    contribution (partition = hidden, so slot r IS hidden block r).
  - The PE idles ~13 us per phase waiting on the collective, which lets
    the HAM clock gate re-throttle it to 1.2 GHz; a run of dummy matmuls
    into a scratch PSUM bank keeps the activity monitor busy so the real
    matmuls run at 2.4 GHz.
  - Output projection is sharded over the contraction dim: each core
    multiplies its local h1T slice by its 128-row slice of out_w and
    writes partial logits [T, B, V]; the host sums the 8 partials.
  - The embedding is folded on the host: R0 = emb @ W0x_slice [256, 512]
    (host also pre-transposes inputs to [T, V, B] bf16).
"""

import numpy as np
import ml_dtypes

import concourse.bass as bass
import concourse.mybir as mybir
import concourse.tile as tile
from concourse.bass_utils import run_bass_kernel_spmd
from concourse.masks import make_identity

T, B, V, E = 256, 128, 256, 512
N0 = N1 = 1024
N_CORES = 8
GS = 128          # per-core slice width of each gate
GW = 4 * GS       # 512 gate columns per core
FP = mybir.dt.float32
BF = mybir.dt.bfloat16

KC0 = N0 // 128   # h K-chunks (8)
KCV = V // 128    # input K-chunks (2)
SLOT = 2 * GS     # 256 cols per receive slot: [h0T chunk | h1T chunk]
N_DUMMY = 40      # keep-warm matmuls per phase (cover the AG window)


def split_excess_waits(nc, limit=1):
    """walrus in this env rejects >1 sem wait per instruction; spill excess
    on_wait entries onto same-engine Nops placed just before the owner."""
    import bass_rust

    for bb in nc.main_func.blocks:
        insts = bb.instructions
        i = 0
        while i < len(insts):
            ins = insts[i]
            si = getattr(ins, "sync_info", None)
            if si is None:
                i += 1
                continue
            waits = list(si.on_wait)
            if len(waits) <= limit:
                i += 1
                continue
            si.on_wait = waits[:limit]
            extra = waits[limit:]
            eng = ins.engine
            new_nops = []
            for s in range(0, len(extra), limit):
                chunk = extra[s : s + limit]
                nop = nc.engines[eng].nop(hint="waitsplit", nofuse=True).ins
                for b2 in nc.main_func.blocks:
                    if b2.instructions and b2.instructions[-1] is nop:
                        b2.instructions.pop()
                        break
                nop.sync_info = bass_rust.SyncInfo(on_wait=chunk, on_update=[])
                new_nops.append(nop)
            insts[i:i] = new_nops
            i += len(new_nops) + 1


def build_nc(t_steps=T):
    nc = bass.Bass("TRN2", target_bir_lowering=False, debug=False)

    inputsT = nc.dram_tensor("inputsT", [t_steps, V, B], BF, kind="ExternalInput")
    r0 = nc.dram_tensor("r0", [V, GW], BF, kind="ExternalInput")        # emb @ W0x slice
    w0h = nc.dram_tensor("w0h", [N0, GW], BF, kind="ExternalInput")
    w1x = nc.dram_tensor("w1x", [N0, GW], BF, kind="ExternalInput")
    w1h = nc.dram_tensor("w1h", [N1, GW], BF, kind="ExternalInput")
    outw = nc.dram_tensor("outw", [GS, V], BF, kind="ExternalInput")    # core's 128 rows
    logits = nc.dram_tensor("logits", [t_steps, B, V], FP, kind="ExternalOutput")

    with tile.TileContext(nc) as tc:
        with (
            tc.tile_pool(name="weights", bufs=1) as wpool,
            tc.tile_pool(name="state", bufs=1) as spool,
            tc.tile_pool(name="hT", bufs=2) as hpool,
            tc.tile_pool(name="work", bufs=3) as work,
            tc.tile_pool(name="snd", bufs=3) as sndpool,
            tc.tile_pool(name="inT", bufs=6) as inpool,
            tc.tile_pool(name="psg", bufs=2, space="PSUM") as psg,
            tc.tile_pool(name="pst", bufs=1, space="PSUM") as pst,
            tc.tile_pool(name="psl", bufs=1, space="PSUM") as psl_pool,
            tc.tile_pool(name="psd", bufs=1, space="PSUM") as psd_pool,
            tc.tile_pool(name="dram", bufs=2, space="DRAM") as dram,
        ):
            # ---- resident weights (bf16) ----
            r0_sb = wpool.tile([128, KCV * GW], BF)        # [128, 2*512]
            for k in range(KCV):
                nc.sync.dma_start(r0_sb[:, k * GW:(k + 1) * GW], r0[k * 128:(k + 1) * 128, :])
            w0h_sb = wpool.tile([128, KC0 * GW], BF)       # [128, 8*512]
            w1x_sb = wpool.tile([128, KC0 * GW], BF)
            w1h_sb = wpool.tile([128, KC0 * GW], BF)
            for (wsb, wdr) in ((w0h_sb, w0h), (w1x_sb, w1x), (w1h_sb, w1h)):
                for k in range(KC0):
                    nc.sync.dma_start(wsb[:, k * GW:(k + 1) * GW], wdr[k * 128:(k + 1) * 128, :])
            outw_sb = wpool.tile([128, V], BF)             # [128, 256]
            nc.sync.dma_start(outw_sb[:], outw[:, :])
            ident = wpool.tile([128, 128], BF)
            make_identity(nc, ident[:])

            # ---- persistent state ----
            c0 = spool.tile([128, GS], FP)   # cell state slices [batch, hid_m]
            c1 = spool.tile([128, GS], FP)
            dummy_ps = psd_pool.tile([128, GW], FP, tag="dummy")

            hT_prev = None

            for t in range(t_steps + 1):
                snd = sndpool.tile([128, SLOT], BF, tag="snd")

                # ---------------- layer 1, step t-1 ----------------
                if t >= 1:
                    ps1 = psg.tile([128, GW], FP, tag="g1")
                    for j in range(KC0):   # x-part: h0T(t-1) @ w1x
                        nc.tensor.matmul(ps1[:], hT_prev[:, j * SLOT:j * SLOT + 128],
                                         w1x_sb[:, j * GW:(j + 1) * GW],
                                         start=(j == 0),
                                         stop=(t == 1 and j == KC0 - 1))
                    if t >= 2:             # h-part: h1T(t-2) @ w1h
                        for j in range(KC0):
                            nc.tensor.matmul(ps1[:], hT_prev[:, j * SLOT + 128:(j + 1) * SLOT],
                                             w1h_sb[:, j * GW:(j + 1) * GW],
                                             start=False, stop=(j == KC0 - 1))
                    _lstm_tail(nc, work, pst, ps1, c1, ident,
                               snd[:, GS:2 * GS], first=(t == 1), lid=1)

                    # out-projection for step t-1 (partial, local slice)
                    pl = psl_pool.tile([128, V], FP, tag="logits")
                    nc.tensor.matmul(pl[:], snd[:, GS:2 * GS], outw_sb[:],
                                     start=True, stop=True)
                    lsb = work.tile([128, V], FP, tag="lsb")
                    nc.vector.tensor_copy(lsb[:], pl[:])
                    nc.sync.dma_start(logits[t - 1, :, :], lsb[:])

                # ---------------- layer 0, step t ----------------
                if t < t_steps:
                    ps0 = psg.tile([128, GW], FP, tag="g0")
                    itile = inpool.tile([128, KCV * 128], BF, tag="inT")
                    src = inputsT[t, :, :].rearrange(
                        "(k q) b -> k q b", k=KCV, q=128).transpose([1, 0, 2])
                    dst = itile[:].rearrange("q (k b) -> q k b", k=KCV, b=128)
                    nc.sync.dma_start(dst, src)
                    for k in range(KCV):
                        nc.tensor.matmul(ps0[:], itile[:, k * 128:(k + 1) * 128],
                                         r0_sb[:, k * GW:(k + 1) * GW],
                                         start=(k == 0),
                                         stop=(t == 0 and k == KCV - 1))
                    if t >= 1:
                        for j in range(KC0):
                            nc.tensor.matmul(ps0[:], hT_prev[:, j * SLOT:j * SLOT + 128],
                                             w0h_sb[:, j * GW:(j + 1) * GW],
                                             start=False, stop=(j == KC0 - 1))
                    _lstm_tail(nc, work, pst, ps0, c0, ident,
                               snd[:, 0:GS], first=(t == 0), lid=0)

                # ---------------- exchange ----------------
                if t < t_steps:
                    ag_in = dram.tile([128, SLOT], BF, tag="agi")
                    ag_out = dram.tile([N_CORES * 128, SLOT], BF, tag="ago")
                    nc.sync.dma_start(ag_in[:], snd[:])
                    nc.gpsimd.collective_compute(
                        "AllGather",
                        mybir.AluOpType.bypass,
                        replica_groups=[list(range(N_CORES))],
                        ins=[ag_in.opt()],
                        outs=[ag_out.opt()],
                    )
                    hT = hpool.tile([128, N_CORES * SLOT], BF, tag="hT")
                    # hT[q, r*256 + c] = ag_out[r*128 + q, c]  (block copy)
                    src = ag_out[:, :].rearrange(
                        "(r q) c -> r q c", r=N_CORES, q=128).transpose([1, 0, 2])
                    dst = hT[:].rearrange("q (r c) -> q r c", r=N_CORES, c=SLOT)
                    nc.sync.dma_start(dst, src)
                    hT_prev = hT

                    # keep the PE's activity monitor busy through the AG
                    # window so real matmuls stay at 2.4 GHz
                    for _ in range(N_DUMMY):
                        nc.tensor.matmul(dummy_ps[:], ident[:], r0_sb[:, 0:GW],
                                         start=True, stop=True)

    split_excess_waits(nc, limit=1)
    return nc


def _lstm_tail(nc, work, pst, ps, c_state, ident, snd_dst, first, lid):
    """gate math for one layer; writes transposed bf16 h-slice into snd_dst."""
    # gate order in the 512 free cols: [f i o g]
    fio = work.tile([128, 3 * GS], FP, tag=f"fio{lid}")
    g = work.tile([128, GS], FP, tag=f"g{lid}")
    nc.scalar.activation(fio[:], ps[:, 0:3 * GS], mybir.ActivationFunctionType.Sigmoid)
    nc.scalar.activation(g[:], ps[:, 3 * GS:4 * GS], mybir.ActivationFunctionType.Tanh)

    if first:
        # c = i * g
        nc.vector.tensor_mul(c_state[:], fio[:, GS:2 * GS], g[:])
    else:
        t1 = work.tile([128, GS], FP, tag=f"t1_{lid}")
        t2 = work.tile([128, GS], FP, tag=f"t2_{lid}")
        nc.vector.tensor_mul(t1[:], fio[:, 0:GS], c_state[:])
        nc.vector.tensor_mul(t2[:], fio[:, GS:2 * GS], g[:])
        nc.vector.tensor_add(c_state[:], t1[:], t2[:])
    tc_t = work.tile([128, GS], FP, tag=f"tc{lid}")
    nc.scalar.activation(tc_t[:], c_state[:], mybir.ActivationFunctionType.Tanh)
    h_sl = work.tile([128, GS], BF, tag=f"h{lid}")
    nc.vector.tensor_mul(h_sl[:], fio[:, 2 * GS:3 * GS], tc_t[:])

    # transpose h slice -> [hid_m, batch] into the combined send tile
    pt = pst.tile([128, 128], BF, tag=f"tr{lid}")
    nc.tensor.transpose(pt[:], h_sl[:], ident[:])
    nc.vector.tensor_copy(snd_dst, pt[:])


_NC_CACHE = {}
LAST_RESULTS = None


def _get_nc(t_steps):
    if t_steps not in _NC_CACHE:
        _NC_CACHE[t_steps] = build_nc(t_steps)
    return _NC_CACHE[t_steps]


def prep_in_maps(inputs, embedding_matrix, lstm_w0, lstm_w1, out_w, t_steps):
    inputs = np.asarray(inputs, np.float32)
    emb = np.asarray(embedding_matrix, np.float32)
    w0 = np.asarray(lstm_w0, np.float32)
    w1 = np.asarray(lstm_w1, np.float32)
    ow = np.asarray(out_w, np.float32)

    bf = ml_dtypes.bfloat16
    inputsT = np.ascontiguousarray(
        inputs[:t_steps].transpose(0, 2, 1)).astype(bf)   # [T, V, B]

    in_maps = []
    for m in range(N_CORES):
        cols = np.concatenate([np.arange(gi * 1024 + m * GS, gi * 1024 + (m + 1) * GS)
                               for gi in range(4)])
        w0s = np.ascontiguousarray(w0[:, cols])           # [1536, 512]
        w1s = np.ascontiguousarray(w1[:, cols])           # [2048, 512]
        r0 = np.ascontiguousarray(emb @ w0s[:E])          # [256, 512]
        in_maps.append({
            "inputsT": inputsT,
            "r0": r0.astype(bf),
            "w0h": np.ascontiguousarray(w0s[E:]).astype(bf),     # [1024, 512]
            "w1x": np.ascontiguousarray(w1s[:N0]).astype(bf),    # [1024, 512]
            "w1h": np.ascontiguousarray(w1s[N0:]).astype(bf),    # [1024, 512]
            "outw": np.ascontiguousarray(ow[m * GS:(m + 1) * GS]).astype(bf),
        })
    return in_maps


def kernel(inputs, embedding_matrix, lstm_w0, lstm_b0, lstm_w1, lstm_b1, out_w, out_b,
           _t_steps=None):
    t_steps = _t_steps or inputs.shape[0]
    assert not np.any(lstm_b0) and not np.any(lstm_b1) and not np.any(out_b), \
        "nonzero biases not supported by this kernel build"

    nc = _get_nc(t_steps)
    in_maps = prep_in_maps(inputs, embedding_matrix, lstm_w0, lstm_w1, out_w, t_steps)

    res = run_bass_kernel_spmd(nc, in_maps, core_ids=list(range(N_CORES)))
    global LAST_RESULTS
    LAST_RESULTS = res
    logits = res.results[0]["logits"].astype(np.float64)
    for m in range(1, N_CORES):
        logits += res.results[m]["logits"]
    return np.ascontiguousarray(
        logits.reshape(t_steps * B, V).astype(np.float32))
